# revision 24
# baseline (speedup 1.0000x reference)
"""KNN graph kernel (DenseDilatedKnnGraph) for Trainium2, 8 NeuronCores.

Problem: x [2, 192, 8192, 1] fp32 -> edge_index [2, 2, 8192, 9] int32.
reference: L2-normalize x along C, pairwise sq-dists over N, top-9 (k=9,
dilation=1) nearest neighbors (indices), stacked with center indices.

Math: for normalized points, ranking by -dist == ranking by cosine
G = Xn^T Xn. Device computes, per query row, the comb-max vector
F1[d] = max(G[q, d], G[q, d + 4096]) (4096 combs of 2 columns) and
ships F1 to the host. Host takes the top-64 combs per row (always
contains every comb holding a true top-9 column: a comb's max is >= the
9th value, and the device/host value skew is the ~4e-3 fp8 input
quantization), rescores the 128 candidate columns exactly in fp64, and
emits jax-top_k order.

Device schedule per 128-query row tile (~4.5 us steady state; ACT and
DVE both ~100% busy -- the two-engine PSUM-evacuation floor):
  - PE: Gram in 8 PSUM eighths [128, 1024] (bufs=4). fp8e4m3
    DoubleRow packs K=192 (zero-padded to 256: two fp8 weights per PE
    cell) into ONE matmul per 512-column chunk -- half the matmul
    passes and PSUM drain traffic of an fp16 version.  Inputs staged
    [128 partitions, 2, cols]: k-subtile 0 = channels 0-127, subtile 1
    = channels 128-191 (partitions 64-127 zero).
  - Eighths j and j+4 are produced back to back; ACT evacuates eighth
    j to fp16 (~1.1us) while DVE folds eighth j+4 directly from PSUM
    against it (tensor_max with one PSUM operand = fused
    evacuate+fold, ~1.2us).  Interleaving the two consumers per pair
    keeps both engines streaming instead of phase-locking on the
    4-deep PSUM pool.
  - DMA: F1 [128, 4096] f16 -> HBM (vout), 1 MB per tile; input DMAs
    split across the sync and gpsimd queues (ACT's queue stays free).
"""

import numpy as np

B = 2
C = 192
N = 8192
NCORES = 8
RBLK = N // 4  # 2048 query rows per core
NT = RBLK // 128  # 16 row tiles per core
NV = 4096  # F1 width; comb(p) = {p, p + 4096}
TCOMB = 64  # combs the host rescores per row

_cache = {}


def _build_nc():
    import concourse.bacc as bacc
    import concourse.mybir as mybir
    from concourse.bass import ts
    from concourse.tile import TileContext

    f32 = mybir.dt.float32
    f16 = mybir.dt.float16
    f8 = mybir.dt.float8e4

    nc = bacc.Bacc("TRN2")

    # fp8 points in DoubleRow layout [partition, k-subtile, col]:
    # [p, 0, n] = channel p, [p, 1, n] = channel 128+p (p<64, else 0).
    xin = nc.dram_tensor("xin", [128, 2, N], f8, kind="ExternalInput")
    wq = nc.dram_tensor("wq", [128, 2, RBLK], f8, kind="ExternalInput")
    vout = nc.dram_tensor("vout", [RBLK, NV], f16, kind="ExternalOutput")

    DCH = 2048  # input DMA chunk

    with TileContext(nc) as tc:
        with (
            tc.tile_pool(name="xpool", bufs=1) as xpool,
            tc.tile_pool(name="gpool", bufs=3) as gpool,
            tc.tile_pool(name="fpool", bufs=3) as fpool,
            tc.tile_pool(name="vpool", bufs=3) as vpool,
            tc.tile_pool(name="gpsum", bufs=4, space="PSUM") as gpsum,
        ):
            wqD = xpool.tile([128, 2, RBLK], f8)
            hD = xpool.tile([128, 2, N], f8)
            # input DMAs spread across engine queues so they run in
            # parallel; the first hD chunk (both k-subtiles) lands
            # first so matmuls start early.
            nc.sync.dma_start(wqD[:, 0, :], wq[:, 0, :])
            nc.gpsimd.dma_start(wqD[:, 1, :], wq[:, 1, :])
            nc.sync.dma_start(hD[:, 0, 0:DCH], xin[:, 0, 0:DCH])
            nc.gpsimd.dma_start(hD[:, 1, 0:DCH], xin[:, 1, 0:DCH])
            for dc in range(1, N // DCH):
                dsl = ts(dc, DCH)
                nc.sync.dma_start(hD[:, 0, dsl], xin[:, 0, dsl])
                nc.gpsimd.dma_start(hD[:, 1, dsl], xin[:, 1, dsl])

            for t in range(NT):
                tsl = ts(t, 128)
                a = {}
                for j in range(4):
                    a[j] = gpool.tile(
                        [128, 1024], f16, tag=f"a{j}", name=f"a{j}"
                    )
                F1 = fpool.tile([128, NV], f16, tag="F1")

                # Eighths j and j+4 are computed together: ACT
                # evacuates eighth j to fp16, DVE folds eighth j+4
                # straight from PSUM against it (comb(d) = {d, d+4096}).
                # Alternating the two consumers per pair keeps ACT and
                # DVE streaming concurrently instead of phase-locking
                # on the 4-deep PSUM pool.
                for pair in range(4):
                    psA = gpsum.tile([128, 1024], f32, tag="ps", name="psA")
                    psB = gpsum.tile([128, 1024], f32, tag="ps", name="psB")
                    for ps, j in ((psA, pair), (psB, pair + 4)):
                        for hh in range(2):
                            csl = ts(2 * j + hh, 512)
                            osl = slice(512 * hh, 512 * hh + 512)
                            nc.tensor.matmul(
                                ps[:, osl], wqD[:, :, tsl], hD[:, :, csl],
                                start=True, stop=True,
                                perf_mode=mybir.MatmulPerfMode.DoubleRow,
                            )
                    nc.scalar.copy(a[pair], psA)
                    nc.vector.tensor_max(
                        F1[:, 1024 * pair : 1024 * (pair + 1)], psB, a[pair]
                    )
                nc.sync.dma_start(vout[tsl, :], F1)

    nc.compile()
    return nc


def _get_nc():
    if "nc" not in _cache:
        _cache["nc"] = _build_nc()
    return _cache["nc"]


def shard_inputs(x):
    """x: [B, C, N, 1] -> 8 per-core inputs: normalized fp8 points in
    DoubleRow layout (full batch) + the core's own query block."""
    import ml_dtypes

    f8 = ml_dtypes.float8_e4m3
    xs = np.ascontiguousarray(np.asarray(x, dtype=np.float32).reshape(B, C, N))
    rns = 1.0 / np.sqrt((xs * xs).sum(axis=1, keepdims=True))  # [B, 1, N]
    xn = xs * rns
    arr = np.zeros((B, 128, 2, N), dtype=f8)
    arr[:, :, 0, :] = xn[:, 0:128, :].astype(f8)
    arr[:, 0:64, 1, :] = xn[:, 128:192, :].astype(f8)
    in_maps = []
    for c in range(NCORES):
        b, r = divmod(c, 4)
        s = r * RBLK
        in_maps.append(
            {
                "xin": arr[b],
                "wq": np.ascontiguousarray(arr[b][:, :, s : s + RBLK]),
            }
        )
    return in_maps


def assemble(results, x):
    """results: 8 dicts with 'vout' [RBLK, NV] f16 comb-max vectors.

    comb(p) = {p + 2048*m : m = 0..3}. Take top-TCOMB combs per row,
    rescore all TCOMB*4 candidate columns with exact fp64 dots of the
    normalized points, and take the true top-8 by (-value, index).
    """
    xs = np.asarray(x, dtype=np.float32).reshape(B, C, N)
    n64 = np.sqrt((xs.astype(np.float64) ** 2).sum(axis=1, keepdims=True))
    xn = np.ascontiguousarray((xs / n64).transpose(0, 2, 1))  # [B, N, C] f64

    nn = np.empty((B, N, 9), np.int32)
    m_off = (np.arange(2, dtype=np.int64) * NV)[None, None, :]
    for c in range(NCORES):
        b, r = divmod(c, 4)
        s = r * RBLK
        V = results[c]["vout"]  # [RBLK, NV] f16
        combs = np.argpartition(-V, TCOMB, axis=1)[:, :TCOMB].astype(np.int64)
        cand = (combs[:, :, None] + m_off).reshape(RBLK, TCOMB * 2)
        rows = np.arange(s, s + RBLK, dtype=np.int64)
        xnb = xn[b]
        top8 = np.empty((RBLK, 8), np.int64)
        CH = 512
        for r0 in range(0, RBLK, CH):
            cc = cand[r0 : r0 + CH]
            rr = rows[r0 : r0 + CH]
            vals = np.einsum("rkc,rc->rk", xnb[cc], xnb[rr], optimize=True)
            vals[cc == rr[:, None]] = -np.inf
            order = np.lexsort((cc, -vals), axis=-1)[:, :8]
            top8[r0 : r0 + CH] = np.take_along_axis(cc, order, axis=1)
        nn[b, s : s + RBLK, 1:9] = top8
        nn[b, s : s + RBLK, 0] = rows
    center = np.broadcast_to(np.arange(N, dtype=np.int32)[None, :, None], (B, N, 9))
    return np.ascontiguousarray(np.stack([nn, center], axis=0).astype(np.int32))


def kernel(x, _trace=False, **trace_kwargs):
    from concourse.bass_utils import run_bass_kernel_spmd

    nc = _get_nc()
    in_maps = shard_inputs(x)
    res = run_bass_kernel_spmd(
        nc, in_maps, core_ids=list(range(NCORES)), trace=_trace, **trace_kwargs
    )
    _cache["last_results"] = res
    return assemble(res.results, x)


# revision 26
# speedup vs baseline: 1.0136x; 1.0136x over previous
"""KNN graph kernel (DenseDilatedKnnGraph) for Trainium2, 8 NeuronCores.

Problem: x [2, 192, 8192, 1] fp32 -> edge_index [2, 2, 8192, 9] int32.
reference: L2-normalize x along C, pairwise sq-dists over N, top-9 (k=9,
dilation=1) nearest neighbors (indices), stacked with center indices.

Math: for normalized points, ranking by -dist == ranking by cosine
G = Xn^T Xn. Device computes, per query row, the comb-max vector
F1[d] = max(G[q, d], G[q, d + 4096]) (4096 combs of 2 columns) and
ships F1 to the host. Host takes the top-64 combs per row (always
contains every comb holding a true top-9 column: a comb's max is >= the
9th value, and the device/host value skew is the ~4e-3 fp8 input
quantization), rescores the 128 candidate columns exactly in fp64, and
emits jax-top_k order.

Device schedule per 128-query row tile (~4.5 us steady state; ACT and
DVE both ~100% busy -- the two-engine PSUM-evacuation floor):
  - PE: Gram in 8 PSUM eighths [128, 1024] (bufs=4). fp8e4m3
    DoubleRow packs K=192 (zero-padded to 256: two fp8 weights per PE
    cell) into ONE matmul per 512-column chunk -- half the matmul
    passes and PSUM drain traffic of an fp16 version.  Inputs staged
    [128 partitions, 2, cols]: k-subtile 0 = channels 0-127, subtile 1
    = channels 128-191 (partitions 64-127 zero).
  - Eighths j and j+4 are produced back to back; ACT evacuates eighth
    j to fp16 (~1.1us) while DVE folds eighth j+4 directly from PSUM
    against it (tensor_max with one PSUM operand = fused
    evacuate+fold, ~1.2us).  Interleaving the two consumers per pair
    keeps both engines streaming instead of phase-locking on the
    4-deep PSUM pool.
  - DMA: F1 [128, 4096] f16 -> HBM (vout), 1 MB per tile; input DMAs
    split across the sync and gpsimd queues (ACT's queue stays free).
"""

import numpy as np

B = 2
C = 192
N = 8192
NCORES = 8
RBLK = N // 4  # 2048 query rows per core
NT = RBLK // 128  # 16 row tiles per core
NV = 4096  # F1 width; comb(p) = {p, p + 4096}
TCOMB = 64  # combs the host rescores per row

_cache = {}


def _build_nc():
    import concourse.bacc as bacc
    import concourse.mybir as mybir
    from concourse.bass import ts
    from concourse.tile import TileContext

    f32 = mybir.dt.float32
    f16 = mybir.dt.float16
    f8 = mybir.dt.float8e4

    nc = bacc.Bacc("TRN2")

    # fp8 points in DoubleRow layout [partition, k-subtile, col]:
    # [p, 0, n] = channel p, [p, 1, n] = channel 128+p (p<64, else 0).
    xin = nc.dram_tensor("xin", [128, 2, N], f8, kind="ExternalInput")
    wq = nc.dram_tensor("wq", [128, 2, RBLK], f8, kind="ExternalInput")
    vout = nc.dram_tensor("vout", [RBLK, NV], f16, kind="ExternalOutput")

    DCH = 2048  # input DMA chunk

    with TileContext(nc) as tc:
        with (
            tc.tile_pool(name="xpool", bufs=1) as xpool,
            tc.tile_pool(name="gpool", bufs=3) as gpool,
            tc.tile_pool(name="fpool", bufs=3) as fpool,
            tc.tile_pool(name="vpool", bufs=3) as vpool,
            tc.tile_pool(name="gpsum", bufs=4, space="PSUM") as gpsum,
        ):
            wqD = xpool.tile([128, 2, RBLK], f8)
            hD = xpool.tile([128, 2, N], f8)
            # input DMAs spread across engine queues so they run in
            # parallel; the first hD chunk (both k-subtiles) lands
            # first so matmuls start early.
            # tile-0 weights and the first 512 columns land first so the
            # first matmul issues as early as possible.
            nc.sync.dma_start(wqD[:, :, 0:128], wq[:, :, 0:128])
            nc.gpsimd.dma_start(hD[:, 0, 0:512], xin[:, 0, 0:512])
            nc.sync.dma_start(hD[:, 1, 0:512], xin[:, 1, 0:512])
            nc.gpsimd.dma_start(wqD[:, :, 128:RBLK], wq[:, :, 128:RBLK])
            nc.sync.dma_start(hD[:, 0, 512:DCH], xin[:, 0, 512:DCH])
            nc.gpsimd.dma_start(hD[:, 1, 512:DCH], xin[:, 1, 512:DCH])
            for dc in range(1, N // DCH):
                dsl = ts(dc, DCH)
                nc.sync.dma_start(hD[:, 0, dsl], xin[:, 0, dsl])
                nc.gpsimd.dma_start(hD[:, 1, dsl], xin[:, 1, dsl])

            for t in range(NT):
                tsl = ts(t, 128)
                a = {}
                for j in range(4):
                    a[j] = gpool.tile(
                        [128, 1024], f16, tag=f"a{j}", name=f"a{j}"
                    )
                F1 = fpool.tile([128, NV], f16, tag="F1")

                # Eighths j and j+4 are computed together: ACT
                # evacuates eighth j to fp16, DVE folds eighth j+4
                # straight from PSUM against it (comb(d) = {d, d+4096}).
                # Alternating the two consumers per pair keeps ACT and
                # DVE streaming concurrently instead of phase-locking
                # on the 4-deep PSUM pool.
                for pair in range(4):
                    psA = gpsum.tile([128, 1024], f32, tag="ps", name="psA")
                    psB = gpsum.tile([128, 1024], f32, tag="ps", name="psB")
                    for ps, j in ((psA, pair), (psB, pair + 4)):
                        for hh in range(2):
                            csl = ts(2 * j + hh, 512)
                            osl = slice(512 * hh, 512 * hh + 512)
                            nc.tensor.matmul(
                                ps[:, osl], wqD[:, :, tsl], hD[:, :, csl],
                                start=True, stop=True,
                                perf_mode=mybir.MatmulPerfMode.DoubleRow,
                            )
                    nc.scalar.copy(a[pair], psA)
                    nc.vector.tensor_max(
                        F1[:, 1024 * pair : 1024 * (pair + 1)], psB, a[pair]
                    )
                    # ship each F1 slice as soon as its fold lands so the
                    # final tile's output drains early
                    q = nc.sync if pair % 2 == 0 else nc.gpsimd
                    q.dma_start(
                        vout[tsl, 1024 * pair : 1024 * (pair + 1)],
                        F1[:, 1024 * pair : 1024 * (pair + 1)],
                    )

    nc.compile()
    return nc


def _get_nc():
    if "nc" not in _cache:
        _cache["nc"] = _build_nc()
    return _cache["nc"]


def shard_inputs(x):
    """x: [B, C, N, 1] -> 8 per-core inputs: normalized fp8 points in
    DoubleRow layout (full batch) + the core's own query block."""
    import ml_dtypes

    f8 = ml_dtypes.float8_e4m3
    xs = np.ascontiguousarray(np.asarray(x, dtype=np.float32).reshape(B, C, N))
    rns = 1.0 / np.sqrt((xs * xs).sum(axis=1, keepdims=True))  # [B, 1, N]
    xn = xs * rns
    arr = np.zeros((B, 128, 2, N), dtype=f8)
    arr[:, :, 0, :] = xn[:, 0:128, :].astype(f8)
    arr[:, 0:64, 1, :] = xn[:, 128:192, :].astype(f8)
    in_maps = []
    for c in range(NCORES):
        b, r = divmod(c, 4)
        s = r * RBLK
        in_maps.append(
            {
                "xin": arr[b],
                "wq": np.ascontiguousarray(arr[b][:, :, s : s + RBLK]),
            }
        )
    return in_maps


def assemble(results, x):
    """results: 8 dicts with 'vout' [RBLK, NV] f16 comb-max vectors.

    comb(p) = {p + 2048*m : m = 0..3}. Take top-TCOMB combs per row,
    rescore all TCOMB*4 candidate columns with exact fp64 dots of the
    normalized points, and take the true top-8 by (-value, index).
    """
    xs = np.asarray(x, dtype=np.float32).reshape(B, C, N)
    n64 = np.sqrt((xs.astype(np.float64) ** 2).sum(axis=1, keepdims=True))
    xn = np.ascontiguousarray((xs / n64).transpose(0, 2, 1))  # [B, N, C] f64

    nn = np.empty((B, N, 9), np.int32)
    m_off = (np.arange(2, dtype=np.int64) * NV)[None, None, :]
    for c in range(NCORES):
        b, r = divmod(c, 4)
        s = r * RBLK
        V = results[c]["vout"]  # [RBLK, NV] f16
        combs = np.argpartition(-V, TCOMB, axis=1)[:, :TCOMB].astype(np.int64)
        cand = (combs[:, :, None] + m_off).reshape(RBLK, TCOMB * 2)
        rows = np.arange(s, s + RBLK, dtype=np.int64)
        xnb = xn[b]
        top8 = np.empty((RBLK, 8), np.int64)
        CH = 512
        for r0 in range(0, RBLK, CH):
            cc = cand[r0 : r0 + CH]
            rr = rows[r0 : r0 + CH]
            vals = np.einsum("rkc,rc->rk", xnb[cc], xnb[rr], optimize=True)
            vals[cc == rr[:, None]] = -np.inf
            order = np.lexsort((cc, -vals), axis=-1)[:, :8]
            top8[r0 : r0 + CH] = np.take_along_axis(cc, order, axis=1)
        nn[b, s : s + RBLK, 1:9] = top8
        nn[b, s : s + RBLK, 0] = rows
    center = np.broadcast_to(np.arange(N, dtype=np.int32)[None, :, None], (B, N, 9))
    return np.ascontiguousarray(np.stack([nn, center], axis=0).astype(np.int32))


def kernel(x, _trace=False, **trace_kwargs):
    from concourse.bass_utils import run_bass_kernel_spmd

    nc = _get_nc()
    in_maps = shard_inputs(x)
    res = run_bass_kernel_spmd(
        nc, in_maps, core_ids=list(range(NCORES)), trace=_trace, **trace_kwargs
    )
    _cache["last_results"] = res
    return assemble(res.results, x)
